# revision 32
# baseline (speedup 1.0000x reference)
"""Multi-head attention (B=4, S=2048, D=1024, H=16, causal) on 8 trn2 cores.

Sharding: core = b*2 + g  (b = batch 0..3, g = head-group 0..1, 8 heads each).
Per core, all matmul operands are bf16 (fp32 PSUM accumulation):
  Q^T/K^T = (Wq/8)^T X^T etc. (d-major layout), V natural (s-major) with an
  appended ones-column per head (computes softmax denominators inside the AV
  matmul), transposed scores S^T = K_h Q_h^T per 128x512 block, causal via
  block skipping + additive -60 triangle mask (bf16 mask matmul accumulated
  into the scores PSUM), head-pair-batched exp on ScalarE, O^T = V'^T P^T
  accumulated over key blocks, normalize via reciprocal_approx_fast (DVE) +
  partition_broadcast (GPSIMD) + fused multiply on the PSUM->SBUF copy,
  final projection O @ Wo_g with the bias folded into the output copy.
Host: input transpose + bf16 cast + shard, and the g-pair partial-sum
(row-parallel Wo all-reduce) at gather time.
"""

import numpy as np

S = 2048
D = 1024
DL = 512          # local head dims per core (8 heads x 64)
HL = 8            # local heads
DK = 64
NB_K = D // 128   # contraction tiles for projections
NB_DB = DL // 128 # d-out blocks
NQ = S // 512     # q blocks
NB_S = S // 128   # s tiles / key blocks
MASK_VAL = -60.0

_NC = {}
_DEBUG = False


def _build_nc():
    import concourse.bass as bass
    import concourse.mybir as mybir
    import concourse.tile as tile
    from concourse import bacc
    from concourse import library_config

    F32 = mybir.dt.float32
    BF16 = mybir.dt.bfloat16
    Exp = mybir.ActivationFunctionType.Exp

    nc = bacc.Bacc(None)

    xqT = nc.dram_tensor("xqT", [D, S], BF16, kind="ExternalInput")
    xkT = nc.dram_tensor("xkT", [D, S], BF16, kind="ExternalInput")
    xvT = nc.dram_tensor("xvT", [D, S], BF16, kind="ExternalInput")
    wq = nc.dram_tensor("wq", [D, DL], BF16, kind="ExternalInput")
    wk = nc.dram_tensor("wk", [D, DL], BF16, kind="ExternalInput")
    wv = nc.dram_tensor("wv", [D, DL], BF16, kind="ExternalInput")
    wo = nc.dram_tensor("wo", [DL, D], BF16, kind="ExternalInput")
    bqs = nc.dram_tensor("bqs", [128, NB_DB], F32, kind="ExternalInput")
    bks = nc.dram_tensor("bks", [128, NB_DB], F32, kind="ExternalInput")
    bvb = nc.dram_tensor("bvb", [128, DL], BF16, kind="ExternalInput")
    bob = nc.dram_tensor("bob", [128, D], F32, kind="ExternalInput")
    trimask = nc.dram_tensor("trimask", [128, 128], BF16, kind="ExternalInput")
    out_d = nc.dram_tensor("out", [S, D], F32, kind="ExternalOutput")
    if _DEBUG:
        dQT = nc.dram_tensor("dQT", [128, S], BF16, kind="ExternalOutput")
        dKT = nc.dram_tensor("dKT", [128, S], BF16, kind="ExternalOutput")
        dVT = nc.dram_tensor("dVT", [128, HL * (DK + 1)], BF16, kind="ExternalOutput")
        dOT = nc.dram_tensor("dOT", [128, S], BF16, kind="ExternalOutput")
        dRC = nc.dram_tensor("dRC", [1, 512], F32, kind="ExternalOutput")
        dRB = nc.dram_tensor("dRB", [64, 512], F32, kind="ExternalOutput")
        dPSO = nc.dram_tensor("dPSO", [128, 512], F32, kind="ExternalOutput")

    with tile.TileContext(nc) as tc, nc.allow_low_precision(
            reason="bf16 matmul operands are intended"):
        with (
            tc.tile_pool(name="const", bufs=1) as cpool,
            tc.tile_pool(name="resident", bufs=1) as rpool,
        ):
            nc.gpsimd.load_library(library_config.attn)

            trimask_sb = cpool.tile([128, 128], BF16, name="trimask", tag="trimask")
            bqs_sb = cpool.tile([128, NB_DB], F32, name="bqs", tag="bqs")
            bks_sb = cpool.tile([128, NB_DB], F32, name="bks", tag="bks")
            bvb_sb = cpool.tile([128, DL], BF16, name="bvb", tag="bvb")
            bob_sb = cpool.tile([128, D], F32, name="bob", tag="bob")

            # resident weights (all bf16)
            wq_sb = rpool.tile([128, NB_K, DL], BF16, name="wq", tag="wq")
            wk_sb = rpool.tile([128, NB_K, DL], BF16, name="wk", tag="wk")
            wv_sb = rpool.tile([128, NB_K, DL], BF16, name="wv", tag="wv")
            wo_sb = rpool.tile([128, NB_DB, D], BF16, name="wo", tag="wo")
            # All weights/consts on the gpsimd DMA queue (k-sliced wq so the
            # first matmul only waits for 128KB); the sync/scalar queues carry
            # only the x-tile streams so they start immediately.
            wq_r = wq.ap().rearrange("(kt p) n -> p kt n", p=128)
            wk_r = wk.ap().rearrange("(kt p) n -> p kt n", p=128)
            for k in range(NB_K):
                nc.gpsimd.dma_start(wq_sb[:, k, :], wq_r[:, k, :])
            nc.gpsimd.dma_start(wk_sb[:], wk_r[:])
            for t, d in [(trimask_sb, trimask),
                         (bqs_sb, bqs), (bks_sb, bks), (bvb_sb, bvb),
                         (bob_sb, bob)]:
                nc.gpsimd.dma_start(t[:], d[:])
            nc.gpsimd.dma_start(
                wv_sb[:], wv.ap().rearrange("(kt p) n -> p kt n", p=128))
            # resident V input; per-q-block column chunks so the first V
            # projection only waits for the first 1MB
            xv_sb = rpool.tile([128, NB_K, S], BF16, name="xv", tag="xv")
            xvT_r = xvT.ap().rearrange("(kt p) n -> p kt n", p=128)
            nc.gpsimd.dma_start(xv_sb[:, :, 0:512], xvT_r[:, :, 0:512])
            nc.gpsimd.dma_start(
                wo_sb[:], wo.ap().rearrange("(kt p) n -> p kt n", p=128))
            for c in range(1, NQ):
                nc.gpsimd.dma_start(xv_sb[:, :, c * 512:(c + 1) * 512],
                                    xvT_r[:, :, c * 512:(c + 1) * 512])

            QT = [rpool.tile([128, S], BF16, name=f"QT{i}", tag=f"QT{i}") for i in range(NB_DB)]
            KT = [rpool.tile([128, S], BF16, name=f"KT{i}", tag=f"KT{i}") for i in range(NB_DB)]
            VT = [rpool.tile([128, HL, DK + 1], BF16, name=f"VT{i}", tag=f"VT{i}")
                  for i in range(NB_S)]
            OT = [rpool.tile([128, S], BF16, name=f"OT{i}", tag=f"OT{i}") for i in range(NB_DB)]

            # ---- Phase A1: Q^T and K^T projections (d-major) ----
            with (
                tc.tile_pool(name="xa", bufs=6) as xpool,
                tc.tile_pool(name="psA", bufs=2, space="PSUM") as pspool,
            ):
                for xT, w_sb, b_sb, dst, dq in [
                    (xqT, wq_sb, bqs_sb, QT, 0),
                    (xkT, wk_sb, bks_sb, KT, 1),
                ]:
                    for s2 in range(NQ // 2):
                        xts = []
                        for k in range(NB_K):
                            xt = xpool.tile([128, 1024], BF16, name=f"x{k % 2}",
                                            tag=f"x{k % 2}")
                            eng = nc.sync if (k + dq) % 2 == 0 else nc.scalar
                            eng.dma_start(
                                xt[:],
                                xT[k * 128:(k + 1) * 128,
                                   s2 * 1024:(s2 + 1) * 1024])
                            xts.append(xt)
                        for sh in range(2):
                            s = s2 * 2 + sh
                            psums = [pspool.tile([128, 512], F32, name=f"pp{db}", tag=f"pp{db}")
                                     for db in range(NB_DB)]
                            for k in range(NB_K):
                                for db in range(NB_DB):
                                    nc.tensor.matmul(
                                        psums[db][:],
                                        w_sb[:, k, db * 128:(db + 1) * 128],
                                        xts[k][:, sh * 512:(sh + 1) * 512],
                                        start=(k == 0), stop=(k == NB_K - 1))
                            for db in range(NB_DB):
                                nc.scalar.add(
                                    dst[db][:, s * 512:(s + 1) * 512],
                                    psums[db][:], b_sb[:, db:db + 1])

            # ---- Phase B: V proj + attention + output proj, per q-block ----
            with (
                tc.tile_pool(name="pt", bufs=4) as ptpool,
                tc.tile_pool(name="nrm", bufs=2) as npool,
                tc.tile_pool(name="osb", bufs=3) as outpool,
                tc.tile_pool(name="psS", bufs=2, space="PSUM") as spool,
                tc.tile_pool(name="psO", bufs=2, space="PSUM") as opool,
            ):
                # head-pair layout inside one 2-bank PSUM tile:
                #   h0 q-window [minq,512) at cols [minq:512]
                #   h1 q-window [minq,512) at cols [512 : 1024-minq]
                # so the live region [minq : 1024-minq] is contiguous and a
                # single exp covers exactly the useful work.
                #
                # Pending AVs are carried in a 2-deep queue ACROSS block
                # boundaries and flushed behind later iterations' matmuls, so
                # the PE never drains waiting on an exp (the exp an AV needs
                # completed two iterations ago). When the flushed entry is its
                # block's final key block, the normalize chain for that head
                # pair is emitted right after its AV.
                pending = []

                def emit_norm(pso_, qb_, pair_):
                    for hi, h in enumerate(pair_):
                        db, base = h // 2, (h % 2) * 64
                        den = npool.tile([1, 512], F32, name="den", tag="den")
                        nc.vector.tensor_copy(den[:], pso_[h][DK:DK + 1, :])
                        rc = npool.tile([1, 512], F32, name="rc", tag="rc")
                        nc.vector.reciprocal_approx_fast(rc[:], den[:])
                        rb = npool.tile([64, 512], F32, name="rb", tag="rb")
                        nc.gpsimd.partition_broadcast(rb[:], rc[:], channels=64)
                        if _DEBUG and hp_dbg(qb_, pair_, hi):
                            nc.sync.dma_start(dRC[:], rc[:])
                            nc.sync.dma_start(dRB[:], rb[:])
                            dps = npool.tile([128, 512], F32, name="dps", tag="dps")
                            nc.vector.tensor_copy(dps[:], pso_[h][:])
                            nc.sync.dma_start(dPSO[:], dps[:])
                        nc.vector.tensor_mul(
                            OT[db][base:base + 64, qb_ * 512:(qb_ + 1) * 512],
                            pso_[h][0:DK, :], rb[:])

                def hp_dbg(qb_, pair_, hi):
                    return qb_ == 0 and pair_[0] == 0 and hi == 0

                def emit_out_proj(qb_):
                    for m in range(4 * qb_, 4 * qb_ + 4):
                        psc = spool.tile([128, 1024], F32, name="s", tag="s")
                        for db in range(NB_DB):
                            for n2 in range(2):
                                nc.tensor.matmul(
                                    psc[:, n2 * 512:(n2 + 1) * 512],
                                    OT[db][:, m * 128:(m + 1) * 128],
                                    wo_sb[:, db, n2 * 512:(n2 + 1) * 512],
                                    start=(db == 0), stop=(db == NB_DB - 1),
                                    skip_group_check=True)
                        ot = outpool.tile([128, 1024], F32, name="ob", tag="ob")
                        nc.vector.tensor_add(ot[:], psc[:], bob_sb[:])
                        nc.sync.dma_start(
                            out_d[m * 128:(m + 1) * 128, :], ot[:])

                def flush_one():
                    if not pending:
                        return
                    kb, pt, minq, pso_, qb_, kbmax_, pair_ = pending.pop(0)
                    for hi, h in enumerate(pair_):
                        src = (slice(minq, 512) if hi == 0
                               else slice(512, 1024 - minq))
                        nc.tensor.matmul(
                            pso_[h][0:DK + 1, minq:512],
                            VT[kb][:, h, :],
                            pt[:, src],
                            start=(kb == 0), stop=(kb == kbmax_ - 1),
                            skip_group_check=True)
                    if kb == kbmax_ - 1:
                        emit_norm(pso_, qb_, pair_)

                def flush_all():
                    while pending:
                        flush_one()

                for qb in range(NQ):
                    # V projection for this q-block's new key rows (fills the
                    # PE while the scalar engine works through the exps)
                    for mi, m in enumerate(range(4 * qb, 4 * qb + 4)):
                        pv = opool.tile([128, 512], F32, name=f"o{m % 2}", tag=f"o{m % 2}")
                        for k in range(NB_K):
                            nc.tensor.matmul(
                                pv[:], xv_sb[:, k, m * 128:(m + 1) * 128],
                                wv_sb[:, k, :],
                                start=(k == 0), stop=(k == NB_K - 1))
                        if mi == 0:
                            flush_all()  # previous q-block's tail AVs + norm
                        nc.vector.memset(VT[m][:, :, DK:DK + 1], 1.0)
                        nc.vector.tensor_add(
                            VT[m][:, :, 0:DK],
                            pv[:].rearrange("p (h c) -> p h c", c=DK),
                            bvb_sb[:].rearrange("p (h c) -> p h c", c=DK))

                    # output projection for the previous q-block's rows
                    if qb > 0:
                        emit_out_proj(qb - 1)

                    for hp in range(HL // 2):
                        pair = (2 * hp, 2 * hp + 1)
                        kbmax = 4 * (qb + 1)
                        pso = {h: opool.tile([128, 512], F32, name=f"o{h % 2}", tag=f"o{h % 2}")
                               for h in pair}

                        for kb in range(kbmax):
                            di = kb - 4 * qb  # >= 0 on diagonal blocks
                            minq = 0 if di < 0 else 128 * min(di, 3)
                            st = spool.tile([128, 1024], F32, name="s", tag="s")
                            for hi, h in enumerate(pair):
                                dst = (slice(minq, 512) if hi == 0
                                       else slice(512, 1024 - minq))
                                db, base = h // 2, (h % 2) * 64
                                nc.tensor.matmul(
                                    st[:, dst],
                                    KT[db][base:base + 64,
                                           kb * 128:(kb + 1) * 128],
                                    QT[db][base:base + 64,
                                           qb * 512 + minq:(qb + 1) * 512],
                                    start=True, stop=True,
                                    skip_group_check=True)
                            pt = ptpool.tile([128, 1024], BF16, name="p", tag="p")
                            nc.scalar.activation(pt[:, minq:1024 - minq],
                                                 st[:, minq:1024 - minq], Exp)
                            if di >= 0:
                                # causal: zero the invalid (q < k) entries of
                                # the diagonal 128-wide block for both heads.
                                # h0's block is at [minq:minq+128); h1's
                                # shifted window puts it at [512:640). The
                                # same p<=j keep-mask applies to both.
                                nc.vector.tensor_mul(
                                    pt[:, minq:minq + 128],
                                    pt[:, minq:minq + 128], trimask_sb[:])
                                nc.vector.tensor_mul(
                                    pt[:, 512:640],
                                    pt[:, 512:640], trimask_sb[:])
                            if len(pending) >= 2:
                                flush_one()
                            pending.append((kb, pt, minq, pso, qb, kbmax, pair))

                flush_all()
                emit_out_proj(NQ - 1)

            if _DEBUG:
                nc.sync.dma_start(dQT[:], QT[0][:])
                nc.sync.dma_start(dKT[:], KT[0][:])
                nc.sync.dma_start(dVT[:], VT[0][:].rearrange("p h c -> p (h c)"))
                nc.sync.dma_start(dOT[:], OT[0][:])

    nc.finalize()
    return nc


def _make_in_maps(query, value, key, Wq, bq, Wk, bk, Wv, bv, Wo, bo):
    import ml_dtypes

    f32 = np.float32
    bf16 = ml_dtypes.bfloat16
    query = np.asarray(query, f32)
    value = np.asarray(value, f32)
    key = np.asarray(key, f32)
    Wq = np.asarray(Wq, f32); bq = np.asarray(bq, f32)
    Wk = np.asarray(Wk, f32); bk = np.asarray(bk, f32)
    Wv = np.asarray(Wv, f32); bv = np.asarray(bv, f32)
    Wo = np.asarray(Wo, f32); bo = np.asarray(bo, f32)

    p = np.arange(128)[:, None]
    j = np.arange(128)[None, :]
    trimask_np = (p <= j).astype(bf16)

    # per-batch transposed inputs (shared by the two head-group cores)
    xT = {}
    for b in range(4):
        xT[b] = (
            np.ascontiguousarray(query[b].T).astype(bf16),
            np.ascontiguousarray(key[b].T).astype(bf16),
            np.ascontiguousarray(value[b].T).astype(bf16),
        )

    in_maps = []
    for b in range(4):
        for g in range(2):
            sl = slice(g * DL, (g + 1) * DL)
            m = {
                "xqT": xT[b][0],
                "xkT": xT[b][1],
                "xvT": xT[b][2],
                "wq": np.ascontiguousarray(Wq[:, sl] / 8.0).astype(bf16),
                "wk": np.ascontiguousarray(Wk[:, sl]).astype(bf16),
                "wv": np.ascontiguousarray(Wv[:, sl]).astype(bf16),
                "wo": np.ascontiguousarray(Wo[sl, :]).astype(bf16),
                "bqs": np.ascontiguousarray((bq[sl] / 8.0).reshape(NB_DB, 128).T),
                "bks": np.ascontiguousarray(bk[sl].reshape(NB_DB, 128).T),
                "bvb": np.ascontiguousarray(
                    np.broadcast_to(bv[sl].reshape(1, DL), (128, DL))).astype(bf16),
                "bob": np.ascontiguousarray(np.broadcast_to(
                    (bo if g == 0 else np.zeros_like(bo)).reshape(1, D),
                    (128, D))),
                "trimask": trimask_np,
            }
            in_maps.append(m)
    return in_maps


def kernel_with_info(inputs, trace=False, reps=1):
    from concourse.bass_utils import run_bass_kernel_spmd

    if 0 not in _NC:
        _NC[0] = _build_nc()

    in_maps = _make_in_maps(**inputs)
    res = run_bass_kernel_spmd(_NC[0], in_maps, core_ids=list(range(8)),
                               trace=trace)
    out = np.empty((4, S, D), np.float32)
    for b in range(4):
        out[b] = res.results[2 * b]["out"] + res.results[2 * b + 1]["out"]
    return out, res


def kernel(**inputs):
    out, _ = kernel_with_info(inputs)
    return out
